# revision 2
# baseline (speedup 1.0000x reference)
"""MOLELinear (mixture-of-linear-experts) Trainium2 kernel, pair-sharded weights.

Math (per group g): out_g = x_g @ (sum_e c[g,e] W_e + W_sh).T + (sum_e c[g,e] b_e + b_sh)

Sharding: data-parallel over the 32 groups -> 4 groups (8192 tokens) per core.
Expert weights are split between HBM-stack partner cores (physical NC pairs
share one HBM stack): each core reads only HALF the out_features of all
expert weights (4.7 MB f32 instead of 9.4 MB), mixes that half for all 8
groups of its PAIR, and the partners exchange the mixed-weight tiles over
cross-NC SBUF->SBUF remote DMA (zero HBM traffic, ~1 MB each way on the
1 TB/s neighbor link). This cuts per-core HBM traffic from ~34.7 MB to
~30 MB.

Per-core local group indexing (host-arranged): locals 0..3 = own groups
(the core's token blocks), locals 4..7 = partner's groups, ordered so that
"my local 4+j" == "partner's local j". The exchange is then fully symmetric
SPMD: every core sends mixed tile (j, h) for j in 4..7 to the partner's
wmixO[(j-4, h)] slot at the same SBUF address. Partner discovery uses
remote_dma_broadcast relative dests (0, 1): the Q7 XORs tpb with its own,
pairing physical XOR-1 neighbors = HBM-stack partners. A tag tile rides
along so the host can verify the realized pairing and transparently rerun
with corrected in_maps if it ever differs.

Out-feature halves: core with half=0 holds global out rows [0,256); half=1
holds [256,512). Core-local out row order is [mine, other]; the host
permutes the bias columns to match and un-permutes the output rows.

Device plan per core (DMA-roofline bound: 16.8 MB x + 4.7 MB w reads f32,
8.4 MB out writes bf16 ~= 87 us at the 358 GB/s per-core HBM limit):
  - w then x stream on the SWDGE queue 0 (f32 HBM -> bf16 SBUF cast inline
    in the DMA datapath); smalls ride the SP HWDGE ring in parallel.
  - Weight mixing split across PE (identity-diagonal matmuls) and DVE
    (fused bf16 scalar_tensor_tensor chains), producing wmixM[(j,h)] bf16
    [128, kl(2) x o_half(256)] tiles.
  - Remote exchange in two Pool-only critical sections (prep-sem handshake
    + trigger_dma(count=N) on SWDGE queue 1).
  - Main GEMM bf16: psum[o128, tok512] = sum_kt w_slice.T @ xT_slice with
    ot 0,1 from wmixM (mine) and ot 2,3 from wmixO (received).
  - Drain on ScalarE: activation(Identity, bias=bmix[:,col]) psum -> bf16;
    per-chunk out DMAs on the ACT HWDGE ring; host exactly expands bf16.
"""
import ml_dtypes
import numpy as np

import concourse.bacc as bacc
import concourse.mybir as mybir
from concourse.alu_op_type import AluOpType
from concourse.tile import TileContext
from concourse.bass_utils import run_bass_kernel_spmd

N_CORES = 8
IN_F = 512
OUT_F = 512
OH = OUT_F // 2                            # 256: my out-feature half
N_EXPERTS = 8
N_GROUPS = 32
TOK_PER_GROUP = 2048
G_PER_CORE = N_GROUPS // N_CORES          # 4 own groups
G_PAIR = 2 * G_PER_CORE                    # 8 locals (own + partner)
TOK_PER_CORE = G_PER_CORE * TOK_PER_GROUP  # 8192
KT = IN_F // 128                           # 4 k-tiles
NE1 = N_EXPERTS + 1                        # experts + shared
F32 = mybir.dt.float32
F32R = mybir.dt.float32r
BF16 = mybir.dt.bfloat16

ECOLS = 2 * OH                             # 512 cols per expert per half
HALF_COLS = NE1 * ECOLS                    # 4608 cols per weight half

PE_MIX = [6, 7, 2, 3]                      # locals mixed on the PE
DVE_MIX = [4, 5, 0, 1]                     # locals mixed on the DVE
SEND = [4, 5, 6, 7]                        # locals sent to the partner
GORDER = [2, 3, 0, 1]                      # GEMM own-group order (mix readiness)


# x streaming schedule: (group, token offset, tokens) — small chunks at the
# front (GEMM starts as soon as the weights finish) and at the very end
# (short serial tail); big chunks in the middle for stream efficiency
def _chunk_plan(gorder):
    plan = []
    for i, g in enumerate(gorder):
        if i == 0:
            plan += [(g, 0, 512), (g, 512, 512), (g, 1024, 1024)]
        elif i < len(gorder) - 1:
            plan.append((g, 0, 2048))
        else:
            plan += [(g, 0, 1024), (g, 1024, 512), (g, 1536, 256), (g, 1792, 256)]
    return plan


_CACHE = {}


def _build():
    nc = bacc.Bacc(trn_type="TRN2", num_swdge_queues=2)
    xT = nc.dram_tensor("xT", (IN_F, TOK_PER_CORE), F32, kind="ExternalInput")
    wt = nc.dram_tensor("wt", (2, 128, HALF_COLS), F32, kind="ExternalInput")
    cb = nc.dram_tensor("cb", (128, G_PAIR * N_EXPERTS), F32, kind="ExternalInput")
    cx = nc.dram_tensor("cx", (NE1, G_PER_CORE), F32R, kind="ExternalInput")
    ball = nc.dram_tensor("ball", (NE1, OUT_F), F32R, kind="ExternalInput")
    ptag = nc.dram_tensor("ptag", (128, 8), F32, kind="ExternalInput")
    outT = nc.dram_tensor("outT", (OUT_F, TOK_PER_CORE), BF16, kind="ExternalOutput")
    ptago = nc.dram_tensor("ptago", (1, 8), F32, kind="ExternalOutput")

    with TileContext(nc) as tc:
        with (
            tc.tile_pool(name="wp", bufs=1) as wp,
            tc.tile_pool(name="mixp", bufs=1) as mixp,
            tc.tile_pool(name="smallp", bufs=1) as smallp,
            tc.tile_pool(name="xp", bufs=4) as xp,
            tc.tile_pool(name="op", bufs=3) as op,
            tc.tile_pool(name="psp", bufs=5, space="PSUM") as psp,
            tc.tile_pool(name="psm", bufs=3, space="PSUM") as psm,
        ):
            # ---- smalls on the SP HWDGE ring (parallel to the SWDGE stream) ----
            cbt = smallp.tile([128, G_PAIR * N_EXPERTS], F32, tag="cb")
            nc.sync.dma_start(cbt[:], cb[:])
            cxt = smallp.tile([NE1, G_PER_CORE], F32R, tag="cx")
            nc.sync.dma_start(cxt[:], cx[:])
            ballt = smallp.tile([NE1, OUT_F], F32R, tag="ball")
            nc.sync.dma_start(ballt[:], ball[:])
            tagM = smallp.tile([128, 8], F32, tag="tagM")
            nc.sync.dma_start(tagM[:], ptag[:])
            tagO = smallp.tile([128, 8], F32, tag="tagO")

            # ---- identity built on-device: iota(q - p) == 0 ----
            iot = smallp.tile([128, 128], F32, tag="iot")
            nc.gpsimd.iota(iot[:], [[1, 128]], base=0, channel_multiplier=-1,
                           allow_small_or_imprecise_dtypes=True)
            identt = smallp.tile([128, 128], BF16, tag="ident")
            nc.vector.tensor_scalar(
                identt[:], iot[:], 0.0, None, AluOpType.is_equal
            )

            # ---- weight halves: SWDGE cast-DMA f32 -> bf16 ----
            wt_ap = wt[:]
            wall = []
            for h in range(2):
                t = wp.tile([128, HALF_COLS], BF16, tag=f"wh{h}")
                nc.gpsimd.dma_start(t[:], wt_ap[h])
                wall.append(t)

            # ---- ci[j,e] = c[j,e] * I on DVE (PE-mixed locals only) ----
            cit = smallp.tile([128, len(PE_MIX) * N_EXPERTS * 128], BF16, tag="ci")
            ci_col = {}
            for jj, j in enumerate(PE_MIX):
                for e in range(N_EXPERTS):
                    i = jj * N_EXPERTS + e
                    ci_col[(j, e)] = i
                    nc.vector.tensor_scalar_mul(
                        cit[:, i * 128 : (i + 1) * 128], identt[:],
                        cbt[:, j * N_EXPERTS + e : j * N_EXPERTS + e + 1],
                    )

            # ---- mixed biases on PE: pb[o128, g] = ballT_slice.T @ cxt ----
            # (own groups only; ball cols already in core-local out order)
            pb = psm.tile([128, 4 * G_PER_CORE], F32, tag="pb", bufs=1)
            for ot in range(4):
                nc.tensor.matmul(
                    pb[:, ot * G_PER_CORE : (ot + 1) * G_PER_CORE],
                    ballt[:, ot * 128 : (ot + 1) * 128],
                    cxt[:],
                    start=True,
                    stop=True,
                )
            bmix = smallp.tile([128, 4 * G_PER_CORE], F32, tag="bmix")
            nc.scalar.copy(bmix[:], pb[:])

            # wmixM[j,h][p, kl*OH+o] = sum_e c[j,e]*W_e^T[(2h+kl)*128+p, o] + sh
            # (o over MY half).  wmixO[g,h] = the other half, received from the
            # partner (its wmixM[(g+4, h)]).
            wmixM = {}
            wmixO = {}
            for j in range(G_PAIR):
                for h in range(2):
                    wmixM[(j, h)] = mixp.tile(
                        [128, ECOLS], BF16, tag=f"wm{j}_{h}", name=f"wm{j}_{h}"
                    )
            for g in range(G_PER_CORE):
                for h in range(2):
                    wmixO[(g, h)] = mixp.tile(
                        [128, ECOLS], BF16, tag=f"wo{g}_{h}", name=f"wo{g}_{h}"
                    )

            def emit_mix_pe(j, h):
                # psum += ci[j,e].T @ W_e half-chunk; shared via unit identity
                w = wall[h]
                pm = psm.tile([128, ECOLS], F32, tag="pm", bufs=2)
                for e in range(N_EXPERTS):
                    nc.tensor.matmul(
                        pm[:],
                        cit[:, ci_col[(j, e)] * 128 : (ci_col[(j, e)] + 1) * 128],
                        w[:, e * ECOLS : (e + 1) * ECOLS],
                        start=(e == 0),
                        stop=False,
                    )
                nc.tensor.matmul(
                    pm[:],
                    identt[:],
                    w[:, N_EXPERTS * ECOLS : NE1 * ECOLS],
                    start=False,
                    stop=True,
                )
                nc.scalar.copy(wmixM[(j, h)][:], pm[:])

            def emit_mix_dve(j, h, acc):
                # fused bf16 STT chain: acc = c0*W0 + Wsh; acc = ce*We + acc
                w = wall[h]
                sh = w[:, N_EXPERTS * ECOLS : NE1 * ECOLS]
                nc.vector.scalar_tensor_tensor(
                    acc[:], w[:, 0:ECOLS],
                    cbt[:, j * N_EXPERTS : j * N_EXPERTS + 1],
                    sh, AluOpType.mult, AluOpType.add,
                )
                for e in range(1, N_EXPERTS):
                    nc.vector.scalar_tensor_tensor(
                        acc[:] if e < N_EXPERTS - 1 else wmixM[(j, h)][:],
                        w[:, e * ECOLS : (e + 1) * ECOLS],
                        cbt[:, j * N_EXPERTS + e : j * N_EXPERTS + e + 1],
                        acc[:], AluOpType.mult, AluOpType.add,
                    )

            # h0 halves for both engines, then h1 halves (w arrival order).
            # Send-destined locals first within each engine's wave so the
            # exchange fires as early as possible.
            acc = mixp.tile([128, ECOLS], BF16, tag="acc")
            for h in range(2):
                for j in PE_MIX:
                    emit_mix_pe(j, h)
                for j in DVE_MIX:
                    emit_mix_dve(j, h, acc)

            # ---- pairwise exchange: SBUF->SBUF remote DMA, zero HBM ----
            rsem = nc.alloc_semaphore("rdma_recv")
            lsem = nc.alloc_semaphore("rdma_local")
            psem = nc.alloc_semaphore("rdma_prep")

            def rdest_slot(i):
                return [None] * i + [(0, 1)] + [None] * (7 - i)

            # crit1: PE-mixed send tiles (locals 6,7) -> partner wmixO[2,3]
            with tc.tile_critical(no_gpsimd_drain=True):
                n = 0
                for j in (6, 7):
                    for h in range(2):
                        nc.gpsimd.remote_dma_broadcast(
                            wmixO[(j - 4, h)][:], wmixM[(j, h)][:],
                            rsem, lsem, rdests=rdest_slot(n), queue_num=1,
                        ).then_inc(psem, 1)
                        n += 1
                nc.gpsimd.wait_ge(psem, 4)
                nc.gpsimd.trigger_dma(count=4, queue_num=1)
                nc.gpsimd.wait_ge(rsem, 8)

            # crit2: DVE-mixed send tiles (locals 4,5) -> partner wmixO[0,1],
            # plus the pairing tag.
            with tc.tile_critical(no_gpsimd_drain=True):
                n = 0
                for j in (4, 5):
                    for h in range(2):
                        nc.gpsimd.remote_dma_broadcast(
                            wmixO[(j - 4, h)][:], wmixM[(j, h)][:],
                            rsem, lsem, rdests=rdest_slot(n), queue_num=1,
                        ).then_inc(psem, 1)
                        n += 1
                nc.gpsimd.remote_dma_broadcast(
                    tagO[:], tagM[:], rsem, lsem,
                    rdests=rdest_slot(4), queue_num=1,
                ).then_inc(psem, 1)
                nc.gpsimd.wait_ge(psem, 9)
                nc.gpsimd.trigger_dma(count=5, queue_num=1)
                nc.gpsimd.wait_ge(rsem, 18)

            nc.sync.dma_start(ptago[:], tagO[0:1, :])

            def wslice(g, kt, ot):
                # [128k, 128o] lhsT slice for own group g, k-tile kt, out-tile ot
                h, kl = kt // 2, kt % 2
                if ot < 2:
                    t = wmixM[(g, h)]
                    return t[:, kl * OH + ot * 128 : kl * OH + ot * 128 + 128]
                t = wmixO[(g, h)]
                return t[:, kl * OH + (ot - 2) * 128 : kl * OH + (ot - 2) * 128 + 128]

            # ---- main GEMM in GORDER ----
            outT_ap = outT[:].rearrange("(ot p) t -> p ot t", p=128)
            for g, toff, ctok in _chunk_plan(GORDER):
                t0 = g * TOK_PER_GROUP + toff
                xs = xp.tile([128, KT * 2048], BF16, tag="x")
                nc.gpsimd.dma_start(
                    xs[:, : KT * ctok].rearrange("p (kt t) -> p kt t", kt=KT),
                    xT[:, t0 : t0 + ctok].rearrange("(kt p) t -> p kt t", p=128),
                )
                oc = op.tile([128, 4 * 2048], BF16, tag="oc")
                for soff in range(0, ctok, 512):
                    sw = min(512, ctok - soff)
                    for ot in range(4):
                        ps = psp.tile([128, 512], F32, tag="ps")
                        for kt in range(KT):
                            nc.tensor.matmul(
                                ps[:, :sw],
                                wslice(g, kt, ot),
                                xs[:, kt * ctok + soff : kt * ctok + soff + sw],
                                start=(kt == 0),
                                stop=(kt == KT - 1),
                            )
                        # drain + per-partition bias add -> bf16
                        nc.scalar.activation(
                            oc[:, ot * ctok + soff : ot * ctok + soff + sw],
                            ps[:, :sw],
                            mybir.ActivationFunctionType.Identity,
                            bias=bmix[:, ot * G_PER_CORE + g : ot * G_PER_CORE + g + 1],
                            scale=1.0,
                        )
                # per-chunk out DMA on the ACT ring
                nc.scalar.dma_start(
                    outT_ap[:, :, t0 : t0 + ctok],
                    oc[:, : 4 * ctok].rearrange("p (ot t) -> p ot t", ot=4),
                )
    nc.finalize()
    return nc


def _pair_of(pairing, c):
    return pairing[c]


def _make_in_maps(x, coefficients, bias_experts, bias_shared, wt_halves, pairing):
    """Build per-core in_maps for a given partner pairing (an involution)."""
    ball_full = np.empty((NE1, OUT_F), np.float32)
    ball_full[:N_EXPERTS] = bias_experts
    ball_full[N_EXPERTS] = bias_shared

    in_maps = []
    for c in range(N_CORES):
        p = pairing[c]
        half = 0 if c < p else 1
        o0 = half * OH
        o1 = OH - o0  # the other half's start

        own = coefficients[c * G_PER_CORE : (c + 1) * G_PER_CORE]      # [4, 8]
        par = coefficients[p * G_PER_CORE : (p + 1) * G_PER_CORE]      # [4, 8]
        cg8 = np.concatenate([own, par], axis=0)                       # [8, 8]
        cb_np = np.broadcast_to(
            cg8.reshape(1, -1), (128, G_PAIR * N_EXPERTS)
        ).copy()
        cx_np = np.empty((NE1, G_PER_CORE), np.float32)
        cx_np[:N_EXPERTS] = own.T
        cx_np[N_EXPERTS] = 1.0

        # bias columns in core-local out order [mine, other]
        ball_np = np.concatenate(
            [ball_full[:, o0 : o0 + OH], ball_full[:, o1 : o1 + OH]], axis=1
        )

        xT_np = np.ascontiguousarray(
            x[c * TOK_PER_CORE : (c + 1) * TOK_PER_CORE].T
        )
        tag_np = np.full((128, 8), float(c), np.float32)
        in_maps.append(
            {
                "xT": xT_np,
                "wt": wt_halves[half],
                "cb": cb_np,
                "cx": cx_np,
                "ball": ball_np,
                "ptag": tag_np,
            }
        )
    return in_maps


def kernel(x, coefficients, weight_experts, bias_experts, weight_shared, bias_shared, sizes):
    x = np.asarray(x)
    coefficients = np.asarray(coefficients)
    weight_experts = np.asarray(weight_experts)
    bias_experts = np.asarray(bias_experts)
    weight_shared = np.asarray(weight_shared)
    bias_shared = np.asarray(bias_shared)

    if "nc" not in _CACHE:
        _CACHE["nc"] = _build()
    nc = _CACHE["nc"]

    # ---- host-side layout prep (no arithmetic) ----
    # wt_halves[half][h, p, e*ECOLS + kl*OH + o] = W_e[half*OH + o, (2h+kl)*128 + p]
    WT = np.concatenate([weight_experts, weight_shared[None]], axis=0).transpose(0, 2, 1)
    wt_halves = []
    for half in range(2):
        o0 = half * OH
        Wh = WT[:, :, o0 : o0 + OH]                      # [9, 512in, 256out]
        wt_halves.append(
            np.ascontiguousarray(
                Wh.reshape(NE1, 2, 2, 128, OH).transpose(1, 3, 0, 2, 4)
            ).reshape(2, 128, HALF_COLS)
        )

    pairing = [c ^ 1 for c in range(N_CORES)]
    in_maps = _make_in_maps(
        x, coefficients, bias_experts, bias_shared, wt_halves, pairing
    )
    res = run_bass_kernel_spmd(nc, in_maps, core_ids=list(range(N_CORES)))

    # Validate the realized physical pairing via the exchanged tags; if the
    # machine pairs cores differently, rebuild inputs for the observed
    # involution and rerun (same NEFF, just different data placement).
    observed = [int(round(float(res.results[c]["ptago"][0, 0]))) for c in range(N_CORES)]
    if observed != pairing:
        ok = all(
            0 <= observed[c] < N_CORES and observed[observed[c]] == c
            and observed[c] != c
            for c in range(N_CORES)
        )
        assert ok, f"remote exchange returned a non-involution pairing: {observed}"
        pairing = observed
        in_maps = _make_in_maps(
            x, coefficients, bias_experts, bias_shared, wt_halves, pairing
        )
        res = run_bass_kernel_spmd(nc, in_maps, core_ids=list(range(N_CORES)))

    # outT [512, 8192] bf16 per core, rows in core-local [mine, other] order
    # -> [8192, 512] f32 (exact expansion), rows back to global order.
    outs = []
    for c in range(N_CORES):
        half = 0 if c < pairing[c] else 1
        o = res.results[c]["outT"].astype(np.float32)    # [512, 8192]
        full = np.empty_like(o)
        o0 = half * OH
        o1 = OH - o0
        full[o0 : o0 + OH] = o[:OH]
        full[o1 : o1 + OH] = o[OH:]
        outs.append(full.T)
    return np.concatenate(outs, axis=0)
